# revision 3
# baseline (speedup 1.0000x reference)
"""CTC loss (sum over batch) on 8 Trainium2 NeuronCores.

Strategy (data-parallel over batch N=64, 8 samples per core):
  - Device streams acts (T=512, 8, L=1024) per core in per-sample chunks of
    128 frames: exp(acts - BIAS) on the ACT engine with free-dim accumulate
    giving the (shifted) softmax denominators s'.
  - GPSIMD indirect_copy gathers the extended-label probabilities G and the
    skip-masked probabilities PM (mask folded into gather indices that point
    at a zero pad column).
  - Gathered tiles bounce through DRAM into a (8 samples, 16*E) free-dim
    layout so the sequential CTC recursion runs on the DVE with all operands
    at partition base 0:   z <- (z + z>>1)*G + (z>>2)*PM
    with per-sample max-rescaling every 8 steps (linear-space stability).
  - Per-sample rows of z are snapshotted at every t in the global union of
    acts_lens-1; host assembles the per-sample costs from tiny DMAed-out
    buffers (snapshots, rescale factors, denominator sums) and returns the
    scalar sum.
"""

import numpy as np
from contextlib import ExitStack

T, N, L, S = 512, 64, 1024, 50
E = 2 * S + 1            # 101 extended-label states
NCORES = 8
NS = N // NCORES         # 8 samples per core
CH = 128                 # frames per sample-chunk
NCHUNK = T // CH         # 4 chunks per sample
W = 16                   # frames per scan window (one bounce read)
RS = 8                   # rescale every RS frames
LP = L + 2               # padded free dim; col L..L+1 hold exp(-inf)=0
EB = E + 2               # z buffer with 2 leading guard columns
BIAS = -7.43             # exp(acts + BIAS): keeps z-growth ~1/step
VMAX = float(2.0 ** 115)  # rescale row max up to ~e^79.7: widens fp32 window
LOG_VMAX = 115.0 * float(np.log(2.0))

_CACHE = {}


def _build_program(snap_steps):
    import concourse.bacc as bacc
    import concourse.tile as tile
    import concourse.mybir as mybir

    dt = mybir.dt
    n_snap = len(snap_steps)
    snap_of = {t: i for i, t in enumerate(snap_steps)}
    NRES = T // RS

    nc = bacc.Bacc("TRN2", target_bir_lowering=False, debug=False,
                   num_devices=NCORES)

    acts_d = nc.dram_tensor("acts", [T, NS, L], dt.float32,
                            kind="ExternalInput").ap()
    idxg_d = nc.dram_tensor("idxg", [NS, 128, 8], dt.uint16,
                            kind="ExternalInput").ap()
    idxm_d = nc.dram_tensor("idxm", [NS, 128, 8], dt.uint16,
                            kind="ExternalInput").ap()
    s_out = nc.dram_tensor("s_out", [128, NCHUNK * NS], dt.float32,
                           kind="ExternalOutput").ap()
    rmax_out = nc.dram_tensor("rmax_out", [NS, NRES], dt.float32,
                              kind="ExternalOutput").ap()
    snap_out = nc.dram_tensor("snap_out", [NS, max(n_snap, 1) * EB],
                              dt.float32, kind="ExternalOutput").ap()
    scr_g = nc.dram_tensor("scr_g", [NS, NCHUNK, CH, E], dt.float32)
    scr_m = nc.dram_tensor("scr_m", [NS, NCHUNK, CH, E], dt.float32)

    with tile.TileContext(nc) as tc, ExitStack() as ctx:
        stream = ctx.enter_context(tc.tile_pool(name="stream", bufs=3))
        gpool = ctx.enter_context(tc.tile_pool(name="gpool", bufs=3))
        wpool = ctx.enter_context(tc.tile_pool(name="wpool", bufs=3))
        const = ctx.enter_context(tc.tile_pool(name="const", bufs=1))

        idxg_t = []
        idxm_t = []
        for n in range(NS):
            ig = const.tile([128, 8], dt.uint16, tag=f"idxg{n}")
            im = const.tile([128, 8], dt.uint16, tag=f"idxm{n}")
            nc.sync.dma_start(ig[:], idxg_d[n])
            nc.sync.dma_start(im[:], idxm_d[n])
            idxg_t.append(ig)
            idxm_t.append(im)

        zb0 = nc.alloc_sbuf_tensor("zb0", [NS, EB], dt.float32).ap()
        zb1 = nc.alloc_sbuf_tensor("zb1", [NS, EB], dt.float32).ap()
        at = nc.alloc_sbuf_tensor("at", [NS, E], dt.float32).ap()
        bt = nc.alloc_sbuf_tensor("bt", [NS, E], dt.float32).ap()
        s_buf = nc.alloc_sbuf_tensor("s_buf", [128, NCHUNK * NS],
                                     dt.float32).ap()
        rmax_b = nc.alloc_sbuf_tensor("rmax_b", [NS, NRES], dt.float32).ap()
        rcp_b = nc.alloc_sbuf_tensor("rcp_b", [NS, 1], dt.float32).ap()
        snap_b = nc.alloc_sbuf_tensor("snap_b", [NS, max(n_snap, 1) * EB],
                                      dt.float32).ap()

        bias_t = nc.alloc_sbuf_tensor("bias_t", [128, 1], dt.float32).ap()
        nc.vector.memset(bias_t[:], BIAS)
        nc.vector.memset(zb0[:], 0.0)
        nc.vector.memset(zb1[:], 0.0)

        # ---- phase A per sample-chunk: stream, exp+accum, gather, bounce out
        for c in range(NCHUNK):
            for n in range(NS):
                a_t = stream.tile([128, LP], dt.float32, tag="a")
                nc.sync.dma_start(a_t[:, 0:L], acts_d[c * CH:(c + 1) * CH, n, :])
                nc.gpsimd.memset(a_t[:, L:LP], -1e30)
                p_t = stream.tile([128, LP], dt.float32, tag="p")
                nc.scalar.activation(
                    p_t[:], a_t[:], mybir.ActivationFunctionType.Exp,
                    bias=bias_t[:], accum_out=s_buf[:, c * NS + n:c * NS + n + 1])
                g_t = gpool.tile([128, E], dt.float32, tag="g")
                m_t = gpool.tile([128, E], dt.float32, tag="m")
                nc.gpsimd.indirect_copy(g_t[:], p_t[:], idxg_t[n][:], True)
                nc.gpsimd.indirect_copy(m_t[:], p_t[:], idxm_t[n][:], True)
                nc.sync.dma_start(scr_g[n, c], g_t[:])
                nc.sync.dma_start(scr_m[n, c], m_t[:])

        # ---- phase B: sequential scan over T in windows of W frames
        zc, zn = zb0, zb1
        nres = 0
        for w in range(T // W):
            c, r0 = (w * W) // CH, (w * W) % CH
            g2 = wpool.tile([NS, W * E], dt.float32, tag="g2")
            m2 = wpool.tile([NS, W * E], dt.float32, tag="m2")
            nc.sync.dma_start(
                g2.rearrange("n (t e) -> n t e", t=W),
                scr_g[:, c, r0:r0 + W, :])
            nc.sync.dma_start(
                m2.rearrange("n (t e) -> n t e", t=W),
                scr_m[:, c, r0:r0 + W, :])
            for dtp in range(W):
                t = w * W + dtp
                P = g2[:, dtp * E:(dtp + 1) * E]
                PM = m2[:, dtp * E:(dtp + 1) * E]
                if t == 0:
                    nc.vector.tensor_copy(zc[:, 2:4], P[:, 0:2])
                else:
                    nc.vector.tensor_add(at[:], zc[:, 2:EB], zc[:, 1:EB - 1])
                    nc.vector.tensor_mul(bt[:], zc[:, 0:E], PM)
                    nc.vector.tensor_mul(at[:], at[:], P)
                    nc.vector.tensor_add(zn[:, 2:EB], at[:], bt[:])
                    zc, zn = zn, zc
                if t in snap_of:
                    i = snap_of[t]
                    nc.vector.tensor_copy(snap_b[:, i * EB:(i + 1) * EB], zc[:])
                if t % RS == RS - 1:
                    nc.vector.reduce_max(rmax_b[:, nres:nres + 1], zc[:, 2:EB],
                                         mybir.AxisListType.X)
                    nc.vector.reciprocal(rcp_b[:], rmax_b[:, nres:nres + 1])
                    nc.vector.tensor_scalar(zc[:, 2:EB], zc[:, 2:EB],
                                            rcp_b[:], VMAX,
                                            mybir.AluOpType.mult,
                                            mybir.AluOpType.mult)
                    nres += 1

        nc.sync.dma_start(s_out, s_buf[:])
        nc.sync.dma_start(rmax_out, rmax_b[:])
        nc.sync.dma_start(snap_out, snap_b[:])

    nc.compile()
    return nc


def _host_prep(labels, acts_lens, labels_lens):
    """ext labels, gather index tiles, snapshot schedule."""
    offs = np.concatenate([[0], np.cumsum(labels_lens)[:-1]]).astype(np.int64)
    idx = np.minimum(offs[:, None] + np.arange(S)[None, :], labels.shape[0] - 1)
    lab = np.where(np.arange(S)[None, :] < labels_lens[:, None], labels[idx], 0)
    ext = np.zeros((N, E), np.int64)
    ext[:, 1::2] = lab
    ext_m2 = np.concatenate([np.full((N, 2), -1), ext[:, :-2]], axis=1)
    can_skip = (ext != 0) & (ext != ext_m2)
    ext_m = np.where(can_skip, ext, L)          # L -> zero pad column

    idxg = np.zeros((N, 128, 8), np.uint16)
    idxm = np.zeros((N, 128, 8), np.uint16)
    for i in range(E):
        p, col = i % 16, i // 16
        for g in range(8):
            idxg[:, 16 * g + p, col] = ext[:, i]
            idxm[:, 16 * g + p, col] = ext_m[:, i]
    snap_steps = sorted(set((np.asarray(acts_lens) - 1).tolist()))
    return ext, idxg, idxm, snap_steps


def kernel(acts, labels, acts_lens, labels_lens):
    from concourse.bass_utils import run_bass_kernel_spmd

    acts = np.ascontiguousarray(np.asarray(acts, np.float32))
    labels = np.asarray(labels, np.int32)
    acts_lens = np.asarray(acts_lens, np.int32)
    labels_lens = np.asarray(labels_lens, np.int32)

    ext, idxg, idxm, snap_steps = _host_prep(labels, acts_lens, labels_lens)
    key = tuple(snap_steps)
    if key not in _CACHE:
        _CACHE[key] = _build_program(snap_steps)
    nc = _CACHE[key]

    in_maps = []
    for core in range(NCORES):
        sl = slice(core * NS, (core + 1) * NS)
        in_maps.append({
            "acts": np.ascontiguousarray(acts[:, sl, :]),
            "idxg": np.ascontiguousarray(idxg[sl]),
            "idxm": np.ascontiguousarray(idxm[sl]),
        })
    res = run_bass_kernel_spmd(nc, in_maps, list(range(NCORES)))

    snap_of = {t: i for i, t in enumerate(snap_steps)}
    total = 0.0
    for core in range(NCORES):
        out = res.results[core]
        s_b = out["s_out"]            # (128, NCHUNK*NS): [t%128, c*NS+n]
        rmax = out["rmax_out"]        # (NS, T//RS)
        snap = out["snap_out"]        # (NS, n_snap*EB)
        for n in range(NS):
            gn = core * NS + n
            tn = int(acts_lens[gn]) - 1
            i = snap_of[tn]
            zrow = snap[n, i * EB:(i + 1) * EB]
            e0 = 2 * int(labels_lens[gn])
            val = float(zrow[2 + e0] + zrow[2 + e0 - 1])
            k = sum(1 for j in range(T // RS) if RS * j + RS - 1 < tn)
            C = float(np.log(rmax[n, :k].astype(np.float64)).sum()) - k * LOG_VMAX
            # s' for t = 0..tn: s'[t] = s_b[t%128, (t//128)*NS+n]
            ts = np.arange(tn + 1)
            sv = s_b[ts % CH, (ts // CH) * NS + n].astype(np.float64)
            ll = np.log(val) + C - np.log(sv).sum()
            total += -ll
    return np.float32(total)
